# revision 6
# baseline (speedup 1.0000x reference)
"""Trainium2 Bass kernel for nn_BatchMinigrid: batched FPV render.

Strategy (per core, 4096 envs):
- Host stages the input grids as 4 pre-rotated, wall-padded 32x32 variants
  (pure data layout, data-independent). The per-env crop+rotation then
  becomes ONE contiguous 597-element slab read whose start index is a
  linear function of (pos, dir), computed on device.
- Indirect DMA gathers one slab per env (128 envs per call).
- closed/open masks computed in env-major layout, transposed to cell-major
  [49 cells x 512 envs] blocks via PE transposes.
- The 5-step visibility fixed point runs as fp32 matmuls with a [113,49]
  conv operator (closed rows + t rows), ACT tanh, DVE masking.
- Final conv in bf16 (sign-exact), mask transposed back, output = mask*crop
  written as int32.
"""
import os
import numpy as np
import ml_dtypes
from contextlib import ExitStack

import concourse.bass as bass
import concourse.tile as tile
from concourse import mybir
from concourse.bass_utils import run_bass_kernel_spmd
from concourse.masks import make_identity

P = 128
NENV = 4096          # envs per core
NCALL = 32           # gather calls per core (128 envs each)
SUP = 8              # supertiles (512 envs each) == matmul blocks
CPS = 4              # gather calls per supertile
EB = 512             # envs per matmul block
SLOT = 640           # slab slot stride (elements), slab run = 597
RUN = 597
VARPIX = NENV * 1024  # pixels per variant per core

LAST_RESULTS = {}    # test harness introspection


# ----------------------------------------------------------------- waitsplit
def _split_excess_waits(nc, limit=1):
    n_split = 0
    for fn in nc.m.functions:
        for blk in fn.blocks:
            insts = blk.instructions
            i = 0
            while i < len(insts):
                inst = insts[i]
                si = getattr(inst, "sync_info", None)
                if si is not None and si.on_wait and len(si.on_wait) > limit:
                    waits = list(si.on_wait)
                    si.on_wait.clear()
                    si.on_wait.extend(waits[-limit:])
                    rest = waits[:-limit]
                    pos = i
                    for j in range(0, len(rest), limit):
                        nop = mybir.InstNoOp(
                            name=f"{inst.name}_wsplit{j}",
                            engine=inst.engine,
                            bass_nofuse=True,
                            sync_info=mybir.SyncInfo(
                                on_wait=rest[j:j + limit], on_update=[]),
                        )
                        insts.insert(pos, nop)
                        pos += 1
                        i += 1
                        n_split += 1
                i += 1
    return n_split


# ----------------------------------------------------------------- builder
def build_nc():
    f32 = mybir.dt.float32
    bf16 = mybir.dt.bfloat16
    i32 = mybir.dt.int32
    nc = bass.Bass()

    var = nc.dram_tensor("var", [4 * VARPIX, 3], bf16, kind="ExternalInput")
    pos = nc.dram_tensor("pos", [NENV, 2], i32, kind="ExternalInput")
    dirs = nc.dram_tensor("dirs", [NENV], i32, kind="ExternalInput")
    lhs_it = nc.dram_tensor("lhs_it", [113, 49], f32, kind="ExternalInput")
    lhs_fin = nc.dram_tensor("lhs_fin", [49, 49], bf16, kind="ExternalInput")
    w27 = nc.dram_tensor("w27", [49, 1], f32, kind="ExternalInput")
    out = nc.dram_tensor("out", [NENV, 147], i32, kind="ExternalOutput")

    AP = bass.AP

    with tile.TileContext(nc) as tc, ExitStack() as ctx:
        const = ctx.enter_context(tc.tile_pool(name="const", bufs=1))
        scal = ctx.enter_context(tc.tile_pool(name="scal", bufs=1))
        slabp = ctx.enter_context(tc.tile_pool(name="slabp", bufs=1))
        workp = ctx.enter_context(tc.tile_pool(name="workp", bufs=2))
        stp = ctx.enter_context(tc.tile_pool(name="stp", bufs=1))
        thp = ctx.enter_context(tc.tile_pool(name="thp", bufs=2))
        outp = ctx.enter_context(tc.tile_pool(name="outp", bufs=1))
        psA = ctx.enter_context(tc.tile_pool(name="psA", bufs=2, space="PSUM"))
        psB = ctx.enter_context(tc.tile_pool(name="psB", bufs=2, space="PSUM"))
        psZ = ctx.enter_context(tc.tile_pool(name="psZ", bufs=2, space="PSUM"))

        # ---------------- constants
        ident = const.tile([P, P], bf16)
        make_identity(nc, ident[:])
        lhs_it_t = const.tile([113, 49], f32)
        nc.sync.dma_start(out=lhs_it_t[:], in_=lhs_it[:])
        lhs_fin_t = const.tile([P, 49], bf16)
        nc.sync.dma_start(out=lhs_fin_t[64:113, :], in_=lhs_fin[:])
        w27_t = const.tile([P, 1], f32)
        nc.sync.dma_start(out=w27_t[64:113, :], in_=w27[:])

        # ---------------- env scalars: e = p*32 + j
        pos_t = scal.tile([P, 64], i32)
        nc.sync.dma_start(out=pos_t[:],
                          in_=pos[:].rearrange("(p j) c -> p (j c)", p=P))
        dir_t = scal.tile([P, 32], i32)
        nc.sync.dma_start(out=dir_t[:],
                          in_=dirs[:].rearrange("(p j) -> p j", p=P))

        def p0v():
            b = pos_t[:]
            return AP(tensor=b.tensor, offset=b.offset, ap=[b.ap[0], [2, 32]])

        def p1v():
            b = pos_t[:]
            return AP(tensor=b.tensor, offset=b.offset + 1, ap=[b.ap[0], [2, 32]])

        TS = nc.vector.tensor_scalar
        TT = nc.vector.tensor_tensor
        Alu = mybir.AluOpType

        with nc.named_scope("scalars"):
            m = []
            for d in range(4):
                md = scal.tile([P, 32], i32, tag=f"m{d}")
                TS(out=md[:], in0=dir_t[:], scalar1=d, scalar2=None,
                   op0=Alu.is_equal)
                m.append(md)
            # d0: r0=2+p1  c0=23-p0 ; d1: r0=26-p0 c0=23-p1
            # d2: r0=26-p1 c0=p0-1  ; d3: r0=2+p0  c0=p1-1
            cand = []
            for expr in [(1, 1, 2), (0, -1, 26), (1, -1, 26), (0, 1, 2),
                         (0, -1, 23), (1, -1, 23), (0, 1, -1), (1, 1, -1)]:
                src, mul, add = expr
                t_ = scal.tile([P, 32], i32, tag=f"cand{len(cand)}")
                TS(out=t_[:], in0=(p1v() if src else p0v()), scalar1=mul,
                   scalar2=add, op0=Alu.mult, op1=Alu.add)
                cand.append(t_)
            r0 = scal.tile([P, 32], i32)
            c0 = scal.tile([P, 32], i32)
            acc_r = scal.tile([P, 32], i32)
            acc_c = scal.tile([P, 32], i32)
            for d in range(4):
                tr = r0 if d == 0 else acc_r
                tcc = c0 if d == 0 else acc_c
                TT(out=tr[:], in0=m[d][:], in1=cand[d][:], op=Alu.mult)
                TT(out=tcc[:], in0=m[d][:], in1=cand[4 + d][:], op=Alu.mult)
                if d > 0:
                    TT(out=r0[:], in0=r0[:], in1=acc_r[:], op=Alu.add)
                    TT(out=c0[:], in0=c0[:], in1=acc_c[:], op=Alu.add)
            kq = scal.tile([P, 32], i32)
            TS(out=kq[:], in0=dir_t[:], scalar1=1, scalar2=VARPIX,
               op0=Alu.add, op1=Alu.mult)
            m3s = scal.tile([P, 32], i32)
            TS(out=m3s[:], in0=m[3][:], scalar1=4 * VARPIX, scalar2=None,
               op0=Alu.mult)
            TT(out=kq[:], in0=kq[:], in1=m3s[:], op=Alu.subtract)
            e_t = scal.tile([P, 32], i32)
            nc.gpsimd.iota(e_t[:], pattern=[[1, 32]], base=0,
                           channel_multiplier=32)
            idx = scal.tile([P, 32], i32)
            TS(out=idx[:], in0=e_t[:], scalar1=1024, scalar2=None, op0=Alu.mult)
            TT(out=idx[:], in0=idx[:], in1=kq[:], op=Alu.add)
            TS(out=r0[:], in0=r0[:], scalar1=32, scalar2=None, op0=Alu.mult)
            TT(out=idx[:], in0=idx[:], in1=r0[:], op=Alu.add)
            TT(out=idx[:], in0=idx[:], in1=c0[:], op=Alu.add)

        NPAIR = SUP // 2
        slabs = [None] * SUP
        sts = [None] * NPAIR
        opens = [None] * NPAIR
        mbs = [None] * NPAIR

        # ---------------- front end per supertile
        def front(s):
            pi, half = divmod(s, 2)
            if half == 0:
                st = stp.tile([P, 2 * EB], f32, tag=f"st{pi}", name=f"st{pi}")
                sts[pi] = st
                op_t = stp.tile([P, 2 * EB], f32, tag=f"open{pi}",
                                name=f"open{pi}")
                opens[pi] = op_t
                nc.scalar.memzero(st[:])
            st = sts[pi]
            op_t = opens[pi]
            hof = half * EB

            slab = slabp.tile([P, CPS * SLOT], bf16, tag=f"slab{s}",
                              name=f"slab{s}")
            slabs[s] = slab
            with nc.named_scope("gather"):
                for g in range(CPS):
                    c = s * CPS + g
                    nc.gpsimd.indirect_dma_start(
                        out=slab[:, g * SLOT: g * SLOT + RUN],
                        out_offset=None,
                        in_=var[:],
                        in_offset=bass.IndirectOffsetOnAxis(
                            ap=idx[:, c:c + 1], axis=0),
                    )

            with nc.named_scope("closed"):
                sb = slab[:]
                def chview(ch):
                    return AP(tensor=sb.tensor, offset=sb.offset + ch,
                              ap=[sb.ap[0], [SLOT, CPS], [96, 7], [3, 7]])
                clA = workp.tile([P, 64 + CPS * 49], bf16, tag="clA",
                                 name=f"clA{s}")
                nc.gpsimd.memset(clA[:, 0:64], 0.0)
                e0 = workp.tile([P, CPS * 49], bf16, tag="e0", name=f"e0{s}")
                TS(out=e0[:].rearrange("p (g x) -> p g x", g=CPS),
                   in0=chview(0), scalar1=2.0, scalar2=None, op0=Alu.is_equal)
                ca = clA[:, 64:]
                TS(out=ca.rearrange("p (g x) -> p g x", g=CPS),
                   in0=chview(2), scalar1=1.0, scalar2=None, op0=Alu.is_equal)
                TT(out=ca, in0=ca, in1=e0[:], op=Alu.max)

            with nc.named_scope("transpose_in"):
                tpA = psA.tile([P, EB], bf16, tag="tpA", name=f"tpA{s}")
                tpB = psB.tile([P, EB], bf16, tag="tpB", name=f"tpB{s}")
                for g in range(CPS):
                    nc.tensor.transpose(
                        out=tpA[0:49, g * P:(g + 1) * P],
                        in_=clA[:, 64 + g * 49: 64 + (g + 1) * 49],
                        identity=ident[:])
                    nc.tensor.transpose(
                        out=tpB[0:113, g * P:(g + 1) * P],
                        in_=clA[:, g * 49: g * 49 + 113],
                        identity=ident[:])
                nc.scalar.copy(out=st[0:49, hof:hof + EB], in_=tpA[0:49, :])
                TS(out=op_t[64:113, hof:hof + EB], in0=tpB[64:113, :],
                   scalar1=-1.0, scalar2=1.0, op0=Alu.mult, op1=Alu.add)
            # t1 = tanh(W[:,27]) * open  (ACT: copy with per-partition scale)
            nc.scalar.activation(
                out=st[64:113, hof:hof + EB],
                in_=op_t[64:113, hof:hof + EB],
                func=mybir.ActivationFunctionType.Copy,
                scale=w27_t[64:113, :])

        # ---------------- iterations per pair
        def iters(pi):
            st = sts[pi]
            op_t = opens[pi]
            for l in range(2, 6):
                with nc.named_scope(f"iter{l}"):
                    zp = psZ.tile([P, 2 * EB], f32, tag="zp", name=f"zp{pi}_{l}")
                    for h in range(2):
                        nc.tensor.matmul(
                            out=zp[64:113, h * EB:(h + 1) * EB],
                            lhsT=lhs_it_t[:],
                            rhs=st[0:113, h * EB:(h + 1) * EB],
                            start=True, stop=True)
                    if l < 5:
                        th = thp.tile([P, 2 * EB], f32, tag="th",
                                      name=f"th{pi}_{l}")
                        nc.scalar.activation(
                            out=th[64:113, :], in_=zp[64:113, :],
                            func=mybir.ActivationFunctionType.Tanh)
                        TT(out=st[64:113, :], in0=th[64:113, :],
                           in1=op_t[64:113, :], op=Alu.mult)
                    else:
                        rl = thp.tile([P, 2 * EB], f32, tag="th",
                                      name=f"rl{pi}")
                        nc.scalar.activation(
                            out=rl[64:113, :], in_=zp[64:113, :],
                            func=mybir.ActivationFunctionType.Relu)
                        mb = stp.tile([P, 2 * EB], bf16, tag=f"mb{pi}",
                                      name=f"mb{pi}")
                        mbs[pi] = mb
                        TT(out=mb[64:113, :], in0=rl[64:113, :],
                           in1=op_t[64:113, :], op=Alu.mult)

        # ---------------- final conv, mask, output per pair
        outbuf = outp.tile([P, NCALL * 147], mybir.dt.int32)

        def final(pi):
            with nc.named_scope("final"):
                zf = psZ.tile([P, 2 * EB], f32, tag="zp", name=f"zf{pi}")
                for h in range(2):
                    nc.tensor.matmul(
                        out=zf[0:49, h * EB:(h + 1) * EB],
                        lhsT=lhs_fin_t[64:113, :],
                        rhs=mbs[pi][64:113, h * EB:(h + 1) * EB],
                        start=True, stop=True)
                mkB = workp.tile([P, 2 * EB], bf16, tag="mkB", name=f"mkB{pi}")
                TS(out=mkB[0:49, :], in0=zf[0:49, :], scalar1=0.0,
                   scalar2=None, op0=Alu.is_gt)
                for half in range(2):
                    s = pi * 2 + half
                    tpM = psB.tile([P, EB], bf16, tag="tpB", name=f"tpM{s}")
                    for g in range(CPS):
                        nc.tensor.transpose(
                            out=tpM[:, g * 64: g * 64 + 49],
                            in_=mkB[0:49, half * EB + g * P: half * EB + (g + 1) * P],
                            identity=ident[0:49, 0:49])
                    mkA = workp.tile([P, CPS * 49], bf16, tag="mkA",
                                     name=f"mkA{s}")
                    tpb_ = tpM[:]
                    nc.scalar.copy(
                        out=mkA[:].rearrange("p (g x) -> p g x", g=CPS),
                        in_=AP(tensor=tpb_.tensor, offset=tpb_.offset,
                               ap=[tpb_.ap[0], [64, CPS], [1, 49]]))
                    ob = outbuf[:]
                    mk = mkA[:]
                    sb = slabs[s][:]
                    for g in range(CPS):
                        c = s * CPS + g
                        out_ap = AP(tensor=ob.tensor,
                                    offset=ob.offset + c * 147,
                                    ap=[ob.ap[0], [21, 7], [3, 7], [1, 3]])
                        crop_ap = AP(tensor=sb.tensor,
                                     offset=sb.offset + g * SLOT,
                                     ap=[sb.ap[0], [96, 7], [3, 7], [1, 3]])
                        mask_ap = AP(tensor=mk.tensor,
                                     offset=mk.offset + g * 49,
                                     ap=[mk.ap[0], [7, 7], [1, 7], [0, 3]])
                        TT(out=out_ap, in0=crop_ap, in1=mask_ap, op=Alu.mult)

        for s in range(SUP):
            front(s)
            if s % 2 == 1:
                iters(s // 2)
                final(s // 2)

        nc.sync.dma_start(
            out=out[:].rearrange("(p c) f -> p c f", p=P),
            in_=outbuf[:].rearrange("p (c f) -> p c f", c=NCALL))

    _split_excess_waits(nc)
    return nc


# ----------------------------------------------------------------- host side
def _conv_matrix(w):
    w = np.asarray(w, np.float32).reshape(3, 3)
    W = np.zeros((49, 49), np.float32)
    for i in range(7):
        for j in range(7):
            for di in (-1, 0, 1):
                for dj in (-1, 0, 1):
                    ii, jj = i + di, j + dj
                    if 0 <= ii < 7 and 0 <= jj < 7:
                        W[i * 7 + j, ii * 7 + jj] = w[di + 1, dj + 1]
    return W


def _variants(g):
    """[n,25,25,3] int32 -> flat [4*n*1024, 3] bf16 (4 rot90s, padded 32x32)."""
    P35 = np.pad(g, ((0, 0), (5, 5), (5, 5), (0, 0)), constant_values=2)
    vs = [np.ascontiguousarray(np.rot90(P35, k, axes=(2, 1))[:, 0:32, 0:32, :])
          for k in range(4)]
    return np.stack(vs).astype(ml_dtypes.bfloat16).reshape(-1, 3)


_NC_CACHE = []


def kernel(grids, agent_pos, agent_dir, weight):
    grids = np.asarray(grids)
    agent_pos = np.ascontiguousarray(np.asarray(agent_pos, np.int32))
    agent_dir = np.ascontiguousarray(np.asarray(agent_dir, np.int32))
    N = grids.shape[0]
    ncores = 8
    per = N // ncores
    assert per == NENV, (N, NENV)

    W = _conv_matrix(weight)
    lhs_it = np.zeros((113, 49), np.float32)
    lhs_it[0:49] = (-0.01 * W).astype(np.float32)    # closed rows
    lhs_it[64:113] = W                               # t rows
    lhs_fin = W.astype(ml_dtypes.bfloat16)
    w27 = np.tanh(W[:, 27]).astype(np.float32).reshape(49, 1)

    in_maps = []
    for c in range(ncores):
        sl = slice(c * per, (c + 1) * per)
        in_maps.append({
            "var": _variants(grids[sl]),
            "pos": agent_pos[sl],
            "dirs": agent_dir[sl],
            "lhs_it": lhs_it,
            "lhs_fin": lhs_fin,
            "w27": w27,
        })

    nc = _NC_CACHE[0] if _NC_CACHE else build_nc()
    if not _NC_CACHE:
        _NC_CACHE.append(nc)

    trace = bool(int(os.environ.get("KERNEL_TRACE", "0")))
    if trace:
        import tracing
        tracing.enable_tracing()
    r = run_bass_kernel_spmd(nc, in_maps, core_ids=list(range(ncores)),
                             trace=trace)
    LAST_RESULTS["bass"] = r
    outs = [res["out"].reshape(per, 7, 7, 3) for res in r.results]
    return np.concatenate(outs, axis=0)
